# revision 35
# baseline (speedup 1.0000x reference)
"""Bass/Tile TRN2 kernel for nn_Loss_46102178955400 (v4).

Loss = CE(train_logits, targets)
     + L1 * sum_gk ||cent_g - memb_gk|| / N_unl
     + L2 * sum_g sum_{k<l} ||memb_gk - memb_gl|| / (K * N_unl)

Sharding: groups (G=512) and CE rows (N_train=4096) split 8 ways.
Each core returns 3 partial sums; host combines.

Device-side strategy (per core):
  - Member rows arrive in transposed chunk layout via
    gpsimd.dma_gather(transpose=True) from a host-padded bf16 copy of
    unlabeled_logits [32768, 1024]: XT[p, c, j] = row_j[c*128+p].
  - Per pair-tile (2 groups, 128 members): PSUM P[128,130] accumulates
    member-member gram (cols 0:128) and member-centroid dots (128:130).
    sq_j comes from the gram diagonal: D = P . (-0.5 I)  (DVE), then
    gpsimd partition_all_reduce sums D over partitions, leaving
    -0.5*sq_j replicated on every partition; its first row feeds three
    rank-1 matmuls that add -0.5 sq_j (row bcast), -0.5 sq_i (col
    bcast), and -0.5 sqc to P, so d2 = -2 P.
  - dm = P * Mneg (Mneg = -2 on valid pair/cent entries, 0 elsewhere);
    four tiles share one dm buffer, one fused ACT sqrt (+accum rowsum)
    per 4-tile block; align partials via a tiny PE matmul over the
    cent-dist columns of the sqrt output.
  - CE: bf16 rows, exp-accum (no max shift; |logit| < 6), ln, minus
    gathered target logits; exp/ln grouped after the loop so the ACT
    table is not thrashed between Sqrt and Exp/Ln.
"""
import sys

sys.path.insert(0, "/opt/trn_rl_repo")

from contextlib import ExitStack

import numpy as np

import concourse.bass as bass
import concourse.bass_isa as bass_isa
import concourse.tile as tile
from concourse import bacc, library_config, mybir
from concourse.ap import AP
from concourse.bass import IndirectOffsetOnAxis
from concourse.bass_utils import run_bass_kernel_spmd
from concourse.masks import make_identity

F32 = mybir.dt.float32
BF16 = mybir.dt.bfloat16
I32 = mybir.dt.int32
I16 = mybir.dt.int16
AF = mybir.ActivationFunctionType
OP = mybir.AluOpType
AX = mybir.AxisListType
RED = bass_isa.ReduceOp

N_CORES = 8
N_TRAIN, N_UNL, C = 4096, 32768, 1000
CP = 1024                 # padded row length (bf16 stride must be %256 bytes)
G, K = 512, 64
GPC = G // N_CORES        # 64 groups per core
RPC = N_TRAIN // N_CORES  # 512 CE rows per core
CE_TILES = RPC // 128     # 4
TILES = GPC // 2          # 32 pair-tiles per core (2 groups each)
GATHERS = 8               # member gathers per core
IDX_PER_GATHER = 512      # 4 tiles per gather
TPG = IDX_PER_GATHER // 128
BLOCKS = TILES // 4       # 8 sqrt blocks
LAMBDA_1, LAMBDA_2 = 1.0, 0.5


def _emit(ctx: ExitStack, tc: tile.TileContext, aps: dict):
    nc = tc.nc
    ul2d, tl2d, ce2d, ceflat = aps["ul2d"], aps["tl2d"], aps["ce2d"], aps["ceflat"]
    midx_d, cidx_d, tidx_d, out_d = (
        aps["midx"], aps["cidx"], aps["tidx"], aps["out"],
    )

    const = ctx.enter_context(tc.tile_pool(name="const", bufs=1))
    xpool = ctx.enter_context(tc.tile_pool(name="xpool", bufs=3))
    gps = ctx.enter_context(tc.tile_pool(name="gps", bufs=6, space="PSUM"))
    sqp = ctx.enter_context(tc.tile_pool(name="sqp", bufs=1, space="PSUM"))
    onep = ctx.enter_context(tc.tile_pool(name="onep", bufs=1, space="PSUM"))
    dpool = ctx.enter_context(tc.tile_pool(name="dpool", bufs=3))
    sqs = ctx.enter_context(tc.tile_pool(name="sqs", bufs=4))
    dmp = ctx.enter_context(tc.tile_pool(name="dmp", bufs=3))
    scp = ctx.enter_context(tc.tile_pool(name="scp", bufs=2))
    cetp = ctx.enter_context(tc.tile_pool(name="cetp", bufs=CE_TILES))
    escp = ctx.enter_context(tc.tile_pool(name="escp", bufs=2))
    sml = ctx.enter_context(tc.tile_pool(name="sml", bufs=2))

    nc.gpsimd.load_library(library_config.mlp)

    # ---- constants ----
    nhI = const.tile([128, 128], F32)      # -0.5 * I
    make_identity(nc, nhI[:])
    i2f = const.tile([128, 128], F32)      # +2 * I (for Mneg diag fix)
    nc.vector.tensor_scalar_mul(i2f[:], nhI[:], 2.0)
    nc.vector.tensor_scalar_mul(nhI[:], nhI[:], -0.5)

    Mneg = const.tile([128, 130], F32)
    nc.vector.memset(Mneg[:], 0.0)
    nc.vector.memset(Mneg[0:64, 0:64], -2.0)
    nc.vector.memset(Mneg[64:128, 64:128], -2.0)
    nc.vector.memset(Mneg[0:64, 128:129], -2.0)
    nc.vector.memset(Mneg[64:128, 129:130], -2.0)
    nc.vector.tensor_tensor(
        out=Mneg[:, 0:128], in0=Mneg[:, 0:128], in1=i2f[:], op=OP.add
    )

    ones1_bf = const.tile([1, 128], BF16)
    nc.vector.memset(ones1_bf[:], 1.0)
    ones130_bf = const.tile([1, 130], BF16)
    nc.vector.memset(ones130_bf[:], 1.0)
    onesc_bf = const.tile([128, 1], BF16)
    nc.vector.memset(onesc_bf[:], 1.0)
    onesc_f = const.tile([128, 1], F32)
    nc.vector.memset(onesc_f[:], 1.0)

    midx = const.tile([128, GATHERS * IDX_PER_GATHER // 16], I16)
    nc.sync.dma_start(out=midx[:], in_=midx_d[:])
    cidx = const.tile([128, 8], I16)
    nc.sync.dma_start(out=cidx[:], in_=cidx_d[:])
    tidx = const.tile([128, CE_TILES], I32)
    nc.sync.dma_start(out=tidx[:], in_=tidx_d[:])

    rsB = const.tile([128, BLOCKS], F32)
    lnr4 = const.tile([128, CE_TILES], F32)
    tv = const.tile([128, CE_TILES], BF16)
    cesub = const.tile([128, CE_TILES], F32)
    fin = const.tile([128, 4], F32)
    nc.vector.memset(fin[:], 0.0)
    sqcm05 = const.tile([1, GPC], BF16)    # -0.5 * ||cent_g||^2 row

    # ---- centroid transposed gather + squared norms ----
    centT = const.tile([128, 8, 128], BF16)
    nc.gpsimd.dma_gather(
        centT[:], tl2d, cidx[:], 128, 128, CP, elem_step=CP, transpose=True,
    )
    cgt = gps.tile([128, 130], F32, tag="P")
    cg = cgt[0:64, 0:64]
    for c in range(8):
        nc.tensor.matmul(
            out=cg, lhsT=centT[:, c, 0:GPC], rhs=centT[:, c, 0:GPC],
            start=(c == 0), stop=(c == 7), skip_group_check=True,
        )
    Dc = sml.tile([64, 64], BF16, tag="Dc")
    nc.vector.tensor_tensor(out=Dc[:], in0=cg, in1=nhI[0:64, 0:64], op=OP.mult)
    sqct = sqp.tile([1, 128], F32, tag="sqr")
    nc.tensor.matmul(
        out=sqct[0:1, 0:64], lhsT=onesc_bf[0:64, 0:1], rhs=Dc[:],
        start=True, stop=True, skip_group_check=True,
    )
    nc.vector.tensor_copy(out=sqcm05[:], in_=sqct[0:1, 0:64])
    aligP = onep.tile([2, 1], F32, tag="alig")

    # ---- CE target-logit gather (one indirect DMA for all 4 tiles) ----
    nc.gpsimd.indirect_dma_start(
        out=tv[:],
        out_offset=None,
        in_=ceflat,
        in_offset=IndirectOffsetOnAxis(ap=tidx[:, 0:CE_TILES], axis=0),
    )

    # ---- main loop ----
    # Software-pipelined so the in-order PE queue never waits mid-chain:
    #   A(t): grams+cent dots (PE), D = P . -0.5I (DVE)
    #   B(t): SQ = partition_all_reduce(D) (Pool) -> -0.5 sq_j on all rows
    #   C(t): three rank-1 matmuls (PE), dm quarter = P . Mneg (DVE)
    #   per 4-tile block: one fused ACT sqrt (+rowsum accum), PE align sum
    st: dict[int, dict] = {}
    blocks: dict[int, dict] = {}
    xts: dict[int, object] = {}
    cets: list = []

    def stageA(t):
        g, tt = divmod(t, TPG)
        xt = xts[g]
        j0 = tt * 128
        P = gps.tile([128, 130], F32, tag="P")
        for c in range(8):
            nc.tensor.matmul(
                out=P[:, 0:128],
                lhsT=xt[:, c, j0 : j0 + 128],
                rhs=xt[:, c, j0 : j0 + 128],
                start=(c == 0), stop=(c == 7), skip_group_check=True,
            )
        for c in range(8):
            nc.tensor.matmul(
                out=P[:, 128:130],
                lhsT=xt[:, c, j0 : j0 + 128],
                rhs=centT[:, c, 2 * t : 2 * t + 2],
                start=(c == 0), stop=(c == 7), skip_group_check=True,
            )
        D = dpool.tile([128, 128], BF16, tag="D")
        nc.vector.tensor_tensor(out=D[:], in0=P[:, 0:128], in1=nhI[:], op=OP.mult)
        st[t] = {"P": P, "D": D}

    def stageB(t):
        s = st[t]
        SQ = sqs.tile([128, 128], BF16, tag="SQ")
        nc.gpsimd.partition_all_reduce(SQ[:], s["D"][:], 128, RED.add)
        s["SQ"] = SQ

    def stageC(t):
        s = st[t]
        P, SQ = s["P"], s["SQ"]
        sqx = SQ[0:1, 0:128]
        nc.tensor.matmul(
            out=P[:, 0:128], lhsT=ones1_bf[:], rhs=sqx,
            start=False, stop=False, skip_group_check=True,
        )
        nc.tensor.matmul(
            out=P[:, 128:130], lhsT=ones1_bf[:],
            rhs=sqcm05[0:1, 2 * t : 2 * t + 2],
            start=False, stop=False, skip_group_check=True,
        )
        nc.tensor.matmul(
            out=P[:, 0:130], lhsT=sqx, rhs=ones130_bf[:],
            start=False, stop=True, skip_group_check=True,
        )
        b, q = divmod(t, 4)
        if q == 0:
            dm4 = dmp.tile([128, 520], F32, tag="dm")
            blocks[b] = {"dm": dm4}
        dm = blocks[b]["dm"]
        nc.vector.tensor_tensor(
            out=dm[:, 130 * q : 130 * (q + 1)], in0=P[:, 0:130], in1=Mneg[:],
            op=OP.mult,
        )
        del st[t]

    def blockSqrt(b):
        dm = blocks[b]["dm"]
        dsc = scp.tile([128, 520], BF16, tag="dsc")
        nc.scalar.activation(
            out=dsc[:], in_=dm[:], func=AF.Sqrt, accum_out=rsB[:, b : b + 1],
        )
        blocks[b]["dsc"] = dsc

    def blockAlign(b):
        dsc = blocks[b]["dsc"]
        for q in range(4):
            nc.tensor.matmul(
                out=aligP[:],
                lhsT=dsc[:, 130 * q + 128 : 130 * q + 130], rhs=onesc_bf[:],
                start=(b == 0 and q == 0),
                stop=(b == BLOCKS - 1 and q == 3),
                skip_group_check=True,
            )

    def emit_gather(g):
        xt = xpool.tile([128, 8, IDX_PER_GATHER], BF16, tag="xt")
        i0 = g * (IDX_PER_GATHER // 16)
        nc.gpsimd.dma_gather(
            xt[:], ul2d, midx[:, i0 : i0 + IDX_PER_GATHER // 16],
            IDX_PER_GATHER, IDX_PER_GATHER, CP, elem_step=CP, transpose=True,
        )
        xts[g] = xt

    for g in range(3):
        emit_gather(g)

    for g in range(GATHERS):
        if g < CE_TILES:
            r0 = g * 128
            cet = cetp.tile([128, C], BF16, tag="cet")
            nc.sync.dma_start(out=cet[:], in_=ce2d[r0 : r0 + 128, 0:C])
            cets.append(cet)

        if g + 3 < GATHERS:
            emit_gather(g + 3)

        if g == 5:
            for cg_ in range(CE_TILES):
                esc = escp.tile([128, C], BF16, tag="esc")
                esum = sml.tile([128, 1], F32, tag="esum")
                nc.scalar.activation(
                    out=esc[:], in_=cets[cg_][:], func=AF.Exp,
                    accum_out=esum[:, 0:1],
                )
                nc.scalar.activation(
                    out=lnr4[:, cg_ : cg_ + 1], in_=esum[:], func=AF.Ln
                )

        for tt in range(TPG):
            t = g * TPG + tt
            stageA(t)
            if t >= 1:
                stageB(t - 1)
            if t >= 3:
                stageC(t - 3)
                if (t - 3) % 4 == 3:
                    b = (t - 3) // 4
                    if b >= 1:
                        blockAlign(b - 1)
                    blockSqrt(b)
    stageB(TILES - 1)
    for t in range(TILES - 3, TILES):
        stageC(t)
    blockAlign(BLOCKS - 2)
    blockSqrt(BLOCKS - 1)
    blockAlign(BLOCKS - 1)

    # ---- final partial sums -> out[1, 8] ----
    nc.vector.tensor_reduce(out=fin[:, 0:1], in_=rsB[:], axis=AX.X, op=OP.add)
    nc.vector.tensor_tensor(out=cesub[:], in0=lnr4[:], in1=tv[:], op=OP.subtract)
    nc.vector.tensor_reduce(out=fin[:, 2:3], in_=cesub[:], axis=AX.X, op=OP.add)
    al_sb = sml.tile([2, 1], F32, tag="al_sb")
    nc.vector.tensor_copy(out=al_sb[:], in_=aligP[:])
    spf = sqp.tile([1, 128], F32, tag="sqr")
    nc.tensor.matmul(
        out=spf[0:1, 0:3], lhsT=onesc_f[:], rhs=fin[:, 0:3],
        start=True, stop=True, skip_group_check=True,
    )
    nc.tensor.matmul(
        out=spf[0:1, 4:5], lhsT=al_sb[:], rhs=onesc_f[0:2, 0:1],
        start=True, stop=True, skip_group_check=True,
    )
    out_sb = sml.tile([1, 8], F32, tag="out_sb")
    nc.vector.memset(out_sb[:], 0.0)
    nc.vector.tensor_copy(out=out_sb[0:1, 0:3], in_=spf[0:1, 0:3])
    nc.vector.tensor_copy(out=out_sb[0:1, 3:4], in_=spf[0:1, 4:5])
    nc.sync.dma_start(out=out_d[:], in_=out_sb[:])


def build_nc():
    nc = bacc.Bacc(
        "TRN2", target_bir_lowering=False, debug=False, num_devices=N_CORES
    )
    ul_t = nc.dram_tensor("ulb", [N_UNL, CP], BF16, kind="ExternalInput")
    tl_t = nc.dram_tensor("tlb", [N_TRAIN, CP], BF16, kind="ExternalInput")
    ce_t = nc.dram_tensor("ce", [RPC, CP], BF16, kind="ExternalInput")
    aps = {
        "ul2d": ul_t.ap(),
        "tl2d": tl_t.ap(),
        "ce2d": ce_t.ap(),
        "ceflat": AP(ce_t.ap().tensor, 0, [[1, RPC * CP], [1, 1]]),
        "midx": nc.dram_tensor(
            "midx", [128, GATHERS * IDX_PER_GATHER // 16], I16,
            kind="ExternalInput",
        ).ap(),
        "cidx": nc.dram_tensor("cidx", [128, 8], I16, kind="ExternalInput").ap(),
        "tidx": nc.dram_tensor(
            "tidx", [128, CE_TILES], I32, kind="ExternalInput"
        ).ap(),
        "out": nc.dram_tensor("out", [1, 8], F32, kind="ExternalOutput").ap(),
    }
    with tile.TileContext(nc) as tc:
        with ExitStack() as ctx:
            _emit(ctx, tc, aps)
    nc.compile()
    return nc


def _wrap16(v: np.ndarray) -> np.ndarray:
    """[n] int16 -> [128, cdiv(n,16)] gather-index layout (i at [i%16, i//16])."""
    n = len(v)
    cols = (n + 15) // 16
    out = np.zeros((128, cols), dtype=np.int16)
    out[:16, :] = v.reshape(cols, 16).T
    return out


def make_in_maps(train_logits, train_targets, unlabeled_logits, centroid_ids,
                 member_ids):
    import ml_dtypes

    tlg = np.asarray(train_logits, dtype=np.float32)
    ulg = np.asarray(unlabeled_logits, dtype=np.float32)
    tgt = np.asarray(train_targets).astype(np.int64)
    cid = np.asarray(centroid_ids).astype(np.int64)
    mid = np.asarray(member_ids).astype(np.int64)

    ulb = np.zeros((N_UNL, CP), dtype=ml_dtypes.bfloat16)
    ulb[:, :C] = ulg.astype(ml_dtypes.bfloat16)
    tlb = np.zeros((N_TRAIN, CP), dtype=ml_dtypes.bfloat16)
    tlb[:, :C] = tlg.astype(ml_dtypes.bfloat16)

    in_maps = []
    for cc in range(N_CORES):
        rows = np.arange(cc * RPC, (cc + 1) * RPC, dtype=np.int64)
        flat = (np.arange(RPC, dtype=np.int64) * CP + tgt[rows]).astype(np.int32)
        tidx = np.ascontiguousarray(flat.reshape(CE_TILES, 128).T)
        gsl = slice(cc * GPC, (cc + 1) * GPC)
        mstream = mid[gsl].reshape(-1).astype(np.int16)     # [4096]
        npg = IDX_PER_GATHER
        midx = np.concatenate(
            [_wrap16(mstream[g * npg : (g + 1) * npg]) for g in range(GATHERS)],
            axis=1,
        )
        cpad = np.zeros(128, dtype=np.int16)
        cpad[:GPC] = cid[gsl].astype(np.int16)
        cidx = _wrap16(cpad)
        in_maps.append({
            "ulb": ulb, "tlb": tlb,
            "ce": np.ascontiguousarray(tlb[cc * RPC : (cc + 1) * RPC]),
            "midx": np.ascontiguousarray(midx),
            "cidx": np.ascontiguousarray(cidx), "tidx": tidx,
        })
    return in_maps


def combine(outs):
    tot = align = ce = 0.0
    for o in outs:
        v = np.asarray(o, dtype=np.float64).reshape(-1)
        tot += v[0]      # 2*pairs + align (rowsums of full dist blocks)
        ce += v[2]
        align += v[3]
    pair2 = tot - align
    ce /= N_TRAIN
    align /= N_UNL
    robust = pair2 / 2.0 / (K * N_UNL)
    return np.float32(ce + LAMBDA_1 * align + LAMBDA_2 * robust)


_NC = None


def _run(in_maps, trace=False):
    global _NC
    if _NC is None:
        _NC = build_nc()
    return run_bass_kernel_spmd(
        _NC, in_maps, list(range(N_CORES)), trace=trace
    )


def kernel(**inputs):
    in_maps = make_in_maps(**inputs)
    res = _run(in_maps)
    return combine([res.results[i]["out"] for i in range(N_CORES)])


# revision 36
# speedup vs baseline: 1.0057x; 1.0057x over previous
"""Bass/Tile TRN2 kernel for nn_Loss_46102178955400 (v4).

Loss = CE(train_logits, targets)
     + L1 * sum_gk ||cent_g - memb_gk|| / N_unl
     + L2 * sum_g sum_{k<l} ||memb_gk - memb_gl|| / (K * N_unl)

Sharding: groups (G=512) and CE rows (N_train=4096) split 8 ways.
Each core returns 3 partial sums; host combines.

Device-side strategy (per core):
  - Member rows arrive in transposed chunk layout via
    gpsimd.dma_gather(transpose=True) from a host-padded bf16 copy of
    unlabeled_logits [32768, 1024]: XT[p, c, j] = row_j[c*128+p].
  - Per pair-tile (2 groups, 128 members): PSUM P[128,130] accumulates
    member-member gram (cols 0:128) and member-centroid dots (128:130).
    sq_j comes from the gram diagonal: D = P . (-0.5 I)  (DVE), then
    gpsimd partition_all_reduce sums D over partitions, leaving
    -0.5*sq_j replicated on every partition; its first row feeds three
    rank-1 matmuls that add -0.5 sq_j (row bcast), -0.5 sq_i (col
    bcast), and -0.5 sqc to P, so d2 = -2 P.
  - dm = P * Mneg (Mneg = -2 on valid pair/cent entries, 0 elsewhere);
    four tiles share one dm buffer, one fused ACT sqrt (+accum rowsum)
    per 4-tile block; align partials via a tiny PE matmul over the
    cent-dist columns of the sqrt output.
  - CE: bf16 rows, exp-accum (no max shift; |logit| < 6), ln, minus
    gathered target logits; exp/ln grouped after the loop so the ACT
    table is not thrashed between Sqrt and Exp/Ln.
"""
import sys

sys.path.insert(0, "/opt/trn_rl_repo")

from contextlib import ExitStack

import numpy as np

import concourse.bass as bass
import concourse.bass_isa as bass_isa
import concourse.tile as tile
from concourse import bacc, library_config, mybir
from concourse.ap import AP
from concourse.bass import IndirectOffsetOnAxis
from concourse.bass_utils import run_bass_kernel_spmd
from concourse.masks import make_identity

F32 = mybir.dt.float32
BF16 = mybir.dt.bfloat16
I32 = mybir.dt.int32
I16 = mybir.dt.int16
AF = mybir.ActivationFunctionType
OP = mybir.AluOpType
AX = mybir.AxisListType
RED = bass_isa.ReduceOp

N_CORES = 8
N_TRAIN, N_UNL, C = 4096, 32768, 1000
CP = 1024                 # padded row length (bf16 stride must be %256 bytes)
G, K = 512, 64
GPC = G // N_CORES        # 64 groups per core
RPC = N_TRAIN // N_CORES  # 512 CE rows per core
CE_TILES = RPC // 128     # 4
TILES = GPC // 2          # 32 pair-tiles per core (2 groups each)
GATHERS = 8               # member gathers per core
IDX_PER_GATHER = 512      # 4 tiles per gather
TPG = IDX_PER_GATHER // 128
BLOCKS = TILES // 4       # 8 sqrt blocks
LAMBDA_1, LAMBDA_2 = 1.0, 0.5


def _emit(ctx: ExitStack, tc: tile.TileContext, aps: dict):
    nc = tc.nc
    ul2d, tl2d, ce2d, ceflat = aps["ul2d"], aps["tl2d"], aps["ce2d"], aps["ceflat"]
    midx_d, cidx_d, tidx_d, out_d = (
        aps["midx"], aps["cidx"], aps["tidx"], aps["out"],
    )

    const = ctx.enter_context(tc.tile_pool(name="const", bufs=1))
    xpool = ctx.enter_context(tc.tile_pool(name="xpool", bufs=3))
    gps = ctx.enter_context(tc.tile_pool(name="gps", bufs=7, space="PSUM"))
    onep = ctx.enter_context(tc.tile_pool(name="onep", bufs=1, space="PSUM"))
    dpool = ctx.enter_context(tc.tile_pool(name="dpool", bufs=3))
    sqs = ctx.enter_context(tc.tile_pool(name="sqs", bufs=4))
    dmp = ctx.enter_context(tc.tile_pool(name="dmp", bufs=3))
    scp = ctx.enter_context(tc.tile_pool(name="scp", bufs=2))
    cetp = ctx.enter_context(tc.tile_pool(name="cetp", bufs=CE_TILES))
    escp = ctx.enter_context(tc.tile_pool(name="escp", bufs=2))
    sml = ctx.enter_context(tc.tile_pool(name="sml", bufs=2))

    nc.gpsimd.load_library(library_config.mlp)

    # ---- constants ----
    nhI = const.tile([128, 128], F32)      # -0.5 * I
    make_identity(nc, nhI[:])
    i2f = const.tile([128, 128], F32)      # +2 * I (for Mneg diag fix)
    nc.vector.tensor_scalar_mul(i2f[:], nhI[:], 2.0)
    nc.vector.tensor_scalar_mul(nhI[:], nhI[:], -0.5)

    Mneg = const.tile([128, 130], F32)
    nc.vector.memset(Mneg[:], 0.0)
    nc.vector.memset(Mneg[0:64, 0:64], -2.0)
    nc.vector.memset(Mneg[64:128, 64:128], -2.0)
    nc.vector.memset(Mneg[0:64, 128:129], -2.0)
    nc.vector.memset(Mneg[64:128, 129:130], -2.0)
    nc.vector.tensor_tensor(
        out=Mneg[:, 0:128], in0=Mneg[:, 0:128], in1=i2f[:], op=OP.add
    )

    ones1_bf = const.tile([1, 128], BF16)
    nc.vector.memset(ones1_bf[:], 1.0)
    ones130_bf = const.tile([1, 130], BF16)
    nc.vector.memset(ones130_bf[:], 1.0)
    onesc_bf = const.tile([128, 1], BF16)
    nc.vector.memset(onesc_bf[:], 1.0)
    onesc_f = const.tile([128, 1], F32)
    nc.vector.memset(onesc_f[:], 1.0)

    midx = const.tile([128, GATHERS * IDX_PER_GATHER // 16], I16)
    nc.sync.dma_start(out=midx[:], in_=midx_d[:])
    cidx = const.tile([128, 8], I16)
    nc.sync.dma_start(out=cidx[:], in_=cidx_d[:])
    tidx = const.tile([128, CE_TILES], I32)
    nc.sync.dma_start(out=tidx[:], in_=tidx_d[:])

    rsB = const.tile([128, BLOCKS], F32)
    lnr4 = const.tile([128, CE_TILES], F32)
    tv = const.tile([128, CE_TILES], BF16)
    cesub = const.tile([128, CE_TILES], F32)
    fin = const.tile([128, 4], F32)
    nc.vector.memset(fin[:], 0.0)
    sqcm05 = const.tile([1, GPC], BF16)    # -0.5 * ||cent_g||^2 row

    # ---- centroid transposed gather + squared norms ----
    centT = const.tile([128, 8, 128], BF16)
    nc.gpsimd.dma_gather(
        centT[:], tl2d, cidx[:], 128, 128, CP, elem_step=CP, transpose=True,
    )
    cgt = gps.tile([128, 130], F32, tag="P")
    cg = cgt[0:64, 0:64]
    for c in range(8):
        nc.tensor.matmul(
            out=cg, lhsT=centT[:, c, 0:GPC], rhs=centT[:, c, 0:GPC],
            start=(c == 0), stop=(c == 7), skip_group_check=True,
        )
    Dc = sml.tile([64, 64], BF16, tag="Dc")
    nc.vector.tensor_tensor(out=Dc[:], in0=cg, in1=nhI[0:64, 0:64], op=OP.mult)
    sqctt = gps.tile([128, 130], F32, tag="P")
    sqct = sqctt[0:1, 0:128]
    nc.tensor.matmul(
        out=sqctt[0:1, 0:64], lhsT=onesc_bf[0:64, 0:1], rhs=Dc[:],
        start=True, stop=True, skip_group_check=True,
    )
    nc.vector.tensor_copy(out=sqcm05[:], in_=sqctt[0:1, 0:64])
    aligP = onep.tile([2, 1], F32, tag="alig")

    # ---- CE target-logit gather (one indirect DMA for all 4 tiles) ----
    nc.gpsimd.indirect_dma_start(
        out=tv[:],
        out_offset=None,
        in_=ceflat,
        in_offset=IndirectOffsetOnAxis(ap=tidx[:, 0:CE_TILES], axis=0),
    )

    # ---- main loop ----
    # Software-pipelined so the in-order PE queue never waits mid-chain:
    #   A(t): grams+cent dots (PE), D = P . -0.5I (DVE)
    #   B(t): SQ = partition_all_reduce(D) (Pool) -> -0.5 sq_j on all rows
    #   C(t): three rank-1 matmuls (PE), dm quarter = P . Mneg (DVE)
    #   per 4-tile block: one fused ACT sqrt (+rowsum accum), PE align sum
    st: dict[int, dict] = {}
    blocks: dict[int, dict] = {}
    xts: dict[int, object] = {}
    cets: list = []

    def stageA(t):
        g, tt = divmod(t, TPG)
        xt = xts[g]
        j0 = tt * 128
        P = gps.tile([128, 130], F32, tag="P")
        for c in range(8):
            nc.tensor.matmul(
                out=P[:, 0:128],
                lhsT=xt[:, c, j0 : j0 + 128],
                rhs=xt[:, c, j0 : j0 + 128],
                start=(c == 0), stop=(c == 7), skip_group_check=True,
            )
        for c in range(8):
            nc.tensor.matmul(
                out=P[:, 128:130],
                lhsT=xt[:, c, j0 : j0 + 128],
                rhs=centT[:, c, 2 * t : 2 * t + 2],
                start=(c == 0), stop=(c == 7), skip_group_check=True,
            )
        D = dpool.tile([128, 128], BF16, tag="D")
        nc.vector.tensor_tensor(out=D[:], in0=P[:, 0:128], in1=nhI[:], op=OP.mult)
        st[t] = {"P": P, "D": D}

    def stageB(t):
        s = st[t]
        SQ = sqs.tile([128, 128], BF16, tag="SQ")
        nc.gpsimd.partition_all_reduce(SQ[:], s["D"][:], 128, RED.add)
        s["SQ"] = SQ

    def stageC(t):
        s = st[t]
        P, SQ = s["P"], s["SQ"]
        sqx = SQ[0:1, 0:128]
        nc.tensor.matmul(
            out=P[:, 0:128], lhsT=ones1_bf[:], rhs=sqx,
            start=False, stop=False, skip_group_check=True,
        )
        nc.tensor.matmul(
            out=P[:, 128:130], lhsT=ones1_bf[:],
            rhs=sqcm05[0:1, 2 * t : 2 * t + 2],
            start=False, stop=False, skip_group_check=True,
        )
        nc.tensor.matmul(
            out=P[:, 0:130], lhsT=sqx, rhs=ones130_bf[:],
            start=False, stop=True, skip_group_check=True,
        )
        b, q = divmod(t, 4)
        if q == 0:
            dm4 = dmp.tile([128, 520], F32, tag="dm")
            blocks[b] = {"dm": dm4}
        dm = blocks[b]["dm"]
        nc.vector.tensor_tensor(
            out=dm[:, 130 * q : 130 * (q + 1)], in0=P[:, 0:130], in1=Mneg[:],
            op=OP.mult,
        )
        del st[t]

    def blockSqrt(b):
        dm = blocks[b]["dm"]
        dsc = scp.tile([128, 520], BF16, tag="dsc")
        nc.scalar.activation(
            out=dsc[:], in_=dm[:], func=AF.Sqrt, accum_out=rsB[:, b : b + 1],
        )
        blocks[b]["dsc"] = dsc

    def blockAlign(b):
        dsc = blocks[b]["dsc"]
        for q in range(4):
            nc.tensor.matmul(
                out=aligP[:],
                lhsT=dsc[:, 130 * q + 128 : 130 * q + 130], rhs=onesc_bf[:],
                start=(b == 0 and q == 0),
                stop=(b == BLOCKS - 1 and q == 3),
                skip_group_check=True,
            )

    def emit_gather(g):
        xt = xpool.tile([128, 8, IDX_PER_GATHER], BF16, tag="xt")
        i0 = g * (IDX_PER_GATHER // 16)
        nc.gpsimd.dma_gather(
            xt[:], ul2d, midx[:, i0 : i0 + IDX_PER_GATHER // 16],
            IDX_PER_GATHER, IDX_PER_GATHER, CP, elem_step=CP, transpose=True,
        )
        xts[g] = xt

    for g in range(3):
        emit_gather(g)

    for g in range(GATHERS):
        if g < CE_TILES:
            r0 = g * 128
            cet = cetp.tile([128, C], BF16, tag="cet")
            nc.sync.dma_start(out=cet[:], in_=ce2d[r0 : r0 + 128, 0:C])
            cets.append(cet)

        if g + 3 < GATHERS:
            emit_gather(g + 3)

        if g == 5:
            for cg_ in range(CE_TILES):
                esc = escp.tile([128, C], BF16, tag="esc")
                esum = sml.tile([128, 1], F32, tag="esum")
                nc.scalar.activation(
                    out=esc[:], in_=cets[cg_][:], func=AF.Exp,
                    accum_out=esum[:, 0:1],
                )
                nc.scalar.activation(
                    out=lnr4[:, cg_ : cg_ + 1], in_=esum[:], func=AF.Ln
                )

        for tt in range(TPG):
            t = g * TPG + tt
            stageA(t)
            if t >= 1:
                stageB(t - 1)
            if t >= 3:
                stageC(t - 3)
                if (t - 3) % 4 == 3:
                    b = (t - 3) // 4
                    if b >= 1:
                        blockAlign(b - 1)
                    blockSqrt(b)
    stageB(TILES - 1)
    for t in range(TILES - 3, TILES):
        stageC(t)
    blockAlign(BLOCKS - 2)
    blockSqrt(BLOCKS - 1)
    blockAlign(BLOCKS - 1)

    # ---- final partial sums -> out[1, 8] ----
    nc.vector.tensor_reduce(out=fin[:, 0:1], in_=rsB[:], axis=AX.X, op=OP.add)
    nc.vector.tensor_tensor(out=cesub[:], in0=lnr4[:], in1=tv[:], op=OP.subtract)
    nc.vector.tensor_reduce(out=fin[:, 2:3], in_=cesub[:], axis=AX.X, op=OP.add)
    al_sb = sml.tile([2, 1], F32, tag="al_sb")
    nc.vector.tensor_copy(out=al_sb[:], in_=aligP[:])
    spft = gps.tile([128, 130], F32, tag="P")
    spf = spft[0:1, 0:128]
    nc.tensor.matmul(
        out=spft[0:1, 0:3], lhsT=onesc_f[:], rhs=fin[:, 0:3],
        start=True, stop=True, skip_group_check=True,
    )
    nc.tensor.matmul(
        out=spft[0:1, 4:5], lhsT=al_sb[:], rhs=onesc_f[0:2, 0:1],
        start=True, stop=True, skip_group_check=True,
    )
    out_sb = sml.tile([1, 8], F32, tag="out_sb")
    nc.vector.memset(out_sb[:], 0.0)
    nc.vector.tensor_copy(out=out_sb[0:1, 0:3], in_=spf[0:1, 0:3])
    nc.vector.tensor_copy(out=out_sb[0:1, 3:4], in_=spf[0:1, 4:5])
    nc.sync.dma_start(out=out_d[:], in_=out_sb[:])


def build_nc():
    nc = bacc.Bacc(
        "TRN2", target_bir_lowering=False, debug=False, num_devices=N_CORES
    )
    ul_t = nc.dram_tensor("ulb", [N_UNL, CP], BF16, kind="ExternalInput")
    tl_t = nc.dram_tensor("tlb", [N_TRAIN, CP], BF16, kind="ExternalInput")
    ce_t = nc.dram_tensor("ce", [RPC, CP], BF16, kind="ExternalInput")
    aps = {
        "ul2d": ul_t.ap(),
        "tl2d": tl_t.ap(),
        "ce2d": ce_t.ap(),
        "ceflat": AP(ce_t.ap().tensor, 0, [[1, RPC * CP], [1, 1]]),
        "midx": nc.dram_tensor(
            "midx", [128, GATHERS * IDX_PER_GATHER // 16], I16,
            kind="ExternalInput",
        ).ap(),
        "cidx": nc.dram_tensor("cidx", [128, 8], I16, kind="ExternalInput").ap(),
        "tidx": nc.dram_tensor(
            "tidx", [128, CE_TILES], I32, kind="ExternalInput"
        ).ap(),
        "out": nc.dram_tensor("out", [1, 8], F32, kind="ExternalOutput").ap(),
    }
    with tile.TileContext(nc) as tc:
        with ExitStack() as ctx:
            _emit(ctx, tc, aps)
    nc.compile()
    return nc


def _wrap16(v: np.ndarray) -> np.ndarray:
    """[n] int16 -> [128, cdiv(n,16)] gather-index layout (i at [i%16, i//16])."""
    n = len(v)
    cols = (n + 15) // 16
    out = np.zeros((128, cols), dtype=np.int16)
    out[:16, :] = v.reshape(cols, 16).T
    return out


def make_in_maps(train_logits, train_targets, unlabeled_logits, centroid_ids,
                 member_ids):
    import ml_dtypes

    tlg = np.asarray(train_logits, dtype=np.float32)
    ulg = np.asarray(unlabeled_logits, dtype=np.float32)
    tgt = np.asarray(train_targets).astype(np.int64)
    cid = np.asarray(centroid_ids).astype(np.int64)
    mid = np.asarray(member_ids).astype(np.int64)

    ulb = np.zeros((N_UNL, CP), dtype=ml_dtypes.bfloat16)
    ulb[:, :C] = ulg.astype(ml_dtypes.bfloat16)
    tlb = np.zeros((N_TRAIN, CP), dtype=ml_dtypes.bfloat16)
    tlb[:, :C] = tlg.astype(ml_dtypes.bfloat16)

    in_maps = []
    for cc in range(N_CORES):
        rows = np.arange(cc * RPC, (cc + 1) * RPC, dtype=np.int64)
        flat = (np.arange(RPC, dtype=np.int64) * CP + tgt[rows]).astype(np.int32)
        tidx = np.ascontiguousarray(flat.reshape(CE_TILES, 128).T)
        gsl = slice(cc * GPC, (cc + 1) * GPC)
        mstream = mid[gsl].reshape(-1).astype(np.int16)     # [4096]
        npg = IDX_PER_GATHER
        midx = np.concatenate(
            [_wrap16(mstream[g * npg : (g + 1) * npg]) for g in range(GATHERS)],
            axis=1,
        )
        cpad = np.zeros(128, dtype=np.int16)
        cpad[:GPC] = cid[gsl].astype(np.int16)
        cidx = _wrap16(cpad)
        in_maps.append({
            "ulb": ulb, "tlb": tlb,
            "ce": np.ascontiguousarray(tlb[cc * RPC : (cc + 1) * RPC]),
            "midx": np.ascontiguousarray(midx),
            "cidx": np.ascontiguousarray(cidx), "tidx": tidx,
        })
    return in_maps


def combine(outs):
    tot = align = ce = 0.0
    for o in outs:
        v = np.asarray(o, dtype=np.float64).reshape(-1)
        tot += v[0]      # 2*pairs + align (rowsums of full dist blocks)
        ce += v[2]
        align += v[3]
    pair2 = tot - align
    ce /= N_TRAIN
    align /= N_UNL
    robust = pair2 / 2.0 / (K * N_UNL)
    return np.float32(ce + LAMBDA_1 * align + LAMBDA_2 * robust)


_NC = None


def _run(in_maps, trace=False):
    global _NC
    if _NC is None:
        _NC = build_nc()
    return run_bass_kernel_spmd(
        _NC, in_maps, list(range(N_CORES)), trace=trace
    )


def kernel(**inputs):
    in_maps = make_in_maps(**inputs)
    res = _run(in_maps)
    return combine([res.results[i]["out"] for i in range(N_CORES)])
